# revision 71
# baseline (speedup 1.0000x reference)
"""MAB (Set-Transformer multihead attention block) Trainium2 Bass kernel, v4.

Reference math (fp32):
  Q = q @ Wq.T + bq ; K = k @ Wk.T + bk ; V = k @ Wv.T + bv    [B,N,256]
  per head h (8 heads x 32): s = Qh @ Kh.T / 16 ; a = softmax(s)
  Oh = Qh + a @ Vh ; o = concat(Oh) ; o = LN0(o) ; o = o + relu(o @ Wo.T + bo)
  out = LN1(o)

Sharding: 8 cores = (batch b in 0..3, query-half in 0..1). Each core handles
1024 queries x 2048 keys of one batch; rows are fully independent through
the whole block (LN/FFN are per-row), so there are no collectives.

Design notes (driven by the TimelineSim cost model; ACT-exp is the
bottleneck engine at ~133us/core and everything else hides under it —
total ~181us/core vs a 457us all-fp32 baseline):
  - score matmuls: direct K=32 row-tiled bf16 contraction (Matmult engine
    time depends only on out-free-rows, not K/M — no head replication).
  - PV runs fp8e4 DoubleRow (0.5 cyc/row, 2 k-tiles contracted per
    instruction; must sit at PE column 0 with M<=64): V_aug carries
    per-head [ones(32) | V(32)] so one accumulation yields the softmax
    denominator (replicated over psum partitions 0:32) AND the numerator
    (32:64). PV goes in two 4-head waves; wave == transposed output half.
  - exp on ACT reads score-PSUM [128, 1024] and writes the fp8 attn tile
    directly; the table is preloaded via a dummy activation; all Q/K/V/W
    matmul inputs are bf16 (fp32r would require fp32r-rounded producers,
    which DMA-fed tensors cannot provide).
  - LN is ACT-free so it never thrashes the exp activation table: stats
    via 1/D-ones matmuls on PE (halves accumulated in psum), per-token
    row math + fast-inverse-sqrt (bit trick + 1 Newton, 0.2% max err) on
    DVE, then K=1 ones-row matmuls broadcast mu/rstd through psum.
    The LN0+FFN+LN1+transpose-out tail of each chunk is emitted after
    the NEXT chunk's scores so chunk boundaries only carry PV wave B.
  - query chunks 3x256 + 2x128 (the short last chunk shrinks the final
    dependency chain); exp instruction size is held at 1024 elements.
  - ALL psum comes from three always-open pools (8 banks total) with
    shared tags, so there is no pool-scope barrier between the preamble
    and the attention loop.
  - ALL input DMAs ride one queue, ordered Wk, Wq, q-half-1, bq/bk,
    k-quarter-1, ... so the first score matmul fires ~10us in; the first
    chunk runs its score groups g-major, interleaved with the remaining
    k-quarter transposes/projections as they land.
"""

import os
import sys
from contextlib import ExitStack

import numpy as np

for _p in ("/opt/trn_rl_repo", "/root/.axon_site/_ro/trn_rl_repo"):
    if os.path.isdir(_p) and _p not in sys.path:
        sys.path.insert(0, _p)

import concourse.bass as bass  # noqa: E402
import concourse.tile as tile  # noqa: E402
from concourse import bacc, bass_isa, mybir  # noqa: E402
from concourse.masks import make_identity  # noqa: E402

F32 = mybir.dt.float32
F32R = mybir.dt.float32r
BF16 = mybir.dt.bfloat16
FP8 = mybir.dt.float8e4
U32 = mybir.dt.uint32
P = 128
EPS = 1e-5

AF = mybir.ActivationFunctionType
OP = mybir.AluOpType
DR = mybir.MatmulPerfMode.DoubleRow
RADD = bass_isa.ReduceOp.add


class Cfg:
    def __init__(self, NQ=1024, NK=2048, D=256, H=8, SC=256):
        self.NQ, self.NK, self.D, self.H = NQ, NK, D, H
        self.HD = D // H            # 32
        self.DO = D // P            # 2
        self.QT = NQ // P
        self.KT = NK // P
        self.SC = min(SC, NQ)       # attention + ln/ffn q-chunk
        self.QCN = NQ // self.SC
        assert self.HD == 32 and self.DO == 2
        assert self.KT % 2 == 0 and self.SC % P == 0


def _emit(nc: bass.Bass, tc: tile.TileContext, ctx: ExitStack, io: dict, cfg: Cfg):
    NQ, NK, D, H = cfg.NQ, cfg.NK, cfg.D, cfg.H
    DO, QT, KT, SC = cfg.DO, cfg.QT, cfg.KT, cfg.SC
    KG = 4                       # score k-tiles per psum tile / exp call

    const = ctx.enter_context(tc.tile_pool(name="const", bufs=1))
    persist = ctx.enter_context(tc.tile_pool(name="persist", bufs=1))
    big = ctx.enter_context(tc.tile_pool(name="big", bufs=2))
    # single set of psum pools for the WHOLE kernel (8 banks):
    #   ps_s : tag "s"  2x[P,KG,SC]f32  (4 banks) scores / transposes
    #   ps_o : tags po0/po1 1x[P,2,SC]f32 each (2 banks) PV accum
    #   ps_f : tag "f"  2x[P,512]f32    (2 banks) projections / FFN / out-T
    ps_s = ctx.enter_context(tc.tile_pool(name="ps_s", bufs=2, space="PSUM"))
    ps_o = ctx.enter_context(tc.tile_pool(name="ps_o", bufs=1, space="PSUM"))
    ps_f = ctx.enter_context(tc.tile_pool(name="ps_f", bufs=2, space="PSUM"))
    attn_pool = ctx.enter_context(tc.tile_pool(name="attn", bufs=1))
    rec_p = ctx.enter_context(tc.tile_pool(name="rec_p", bufs=2))
    ln_p = ctx.enter_context(tc.tile_pool(name="ln_p", bufs=2))

    # ---- input DMAs first. The DMA engines are a single shared resource
    # (~330 GB/s effective), so ORDER is what matters: small weights and q
    # first (unblocks the Q-side pipeline), then k in halves (k_T transposes
    # of the first half overlap the second half's transfer), then Wv/Wo.
    # ALL input DMAs ride one queue (SP) so the shared DMA engine serves
    # them in exactly this order: Wk, Wq, first k quarter, first q half,
    # biases, Wv, k2, q2, k3, k4, Wo. The first score matmul needs only
    # {Wk, k quarter 1, Wq, q half 1}.
    ph0_cm = tc.tile_pool(name="ph0", bufs=1)
    ph0 = ph0_cm.__enter__()
    w_nat = {}

    def load_w_nat(name):
        w_nat[name] = ph0.tile([P, DO, D], F32, name=f"{name}_nat")
        nc.sync.dma_start(w_nat[name],
                          io[name][:].rearrange("(o p) f -> p o f", p=P))

    def vec_pm(name):
        t = const.tile([P, DO], F32, name=f"{name}_pm")
        nc.sync.dma_start(t, io[name][:].rearrange("(o p) -> p o", p=P))
        return t

    load_w_nat("Wk")
    load_w_nat("Wq")
    q_sb = ph0.tile([P, QT, D], F32, name="q_nat")
    qio = io["q"][:].rearrange("(t p) d -> p t d", p=P)
    nc.sync.dma_start(q_sb[:, :QT // 2, :], qio[:, :QT // 2, :])
    bq_pm, bk_pm = vec_pm("bq"), vec_pm("bk")
    k_sb = ph0.tile([P, KT, D], F32, name="k_nat")
    kio = io["k"][:].rearrange("(t p) d -> p t d", p=P)
    nc.sync.dma_start(k_sb[:, 0:4, :], kio[:, 0:4, :])

    bv_row = const.tile([1, D], F32, name="bv_row")
    nc.sync.dma_start(bv_row, io["bv"][:].rearrange("(o d) -> o d", o=1))
    bo_pm = vec_pm("bo")
    g0_pm, b0_pm = vec_pm("g0"), vec_pm("b0")
    g1_pm, b1_pm = vec_pm("g1"), vec_pm("b1")
    load_w_nat("Wv")
    nc.sync.dma_start(k_sb[:, 4:8, :], kio[:, 4:8, :])
    nc.sync.dma_start(q_sb[:, QT // 2:, :], qio[:, QT // 2:, :])
    for c in range(2, KT // 4):
        nc.sync.dma_start(k_sb[:, 4 * c:4 * (c + 1), :],
                          kio[:, 4 * c:4 * (c + 1), :])
    load_w_nat("Wo")

    # ---- constants ----
    ident = const.tile([P, P], F32)
    make_identity(nc, ident)
    ident_bf = const.tile([P, P], BF16)
    nc.vector.tensor_copy(ident_bf, ident)
    zerot = const.tile([P, 1], F32)
    nc.vector.memset(zerot, 0.0)
    sumw_bf = const.tile([P, 1], BF16)   # 1/D: stats matmuls yield means
    nc.vector.memset(sumw_bf, 1.0 / D)
    ones_row = const.tile([1, P], F32)
    nc.vector.memset(ones_row, 1.0)
    c_magic = const.tile([1, SC], U32)
    nc.vector.memset(c_magic, 0x5F3759DF)
    # preload the exp activation table while ACT is otherwise idle
    scratch1 = const.tile([P, 1], F32)
    nc.scalar.activation(scratch1, zerot, AF.Exp)

    def transpose_f32(ps, src):
        nc.tensor.matmul(ps, lhsT=src, rhs=ident[:], is_transpose=True)

    # ---- phase 0: weight/input transposes + projections ----
    # V_aug per head h: 64 fp8 columns = [ones(32) | V(32)]. The DoubleRow
    # PV (which must sit at PE column 0, M<=64) then yields the softmax
    # denominator on psum partitions 0:32 and the numerator on 32:64.
    Q_bf = persist.tile([P, DO, NQ], BF16, name="Q_bf")
    K_bf = persist.tile([P, DO, NK], BF16, name="K_bf")
    V_aug = persist.tile([P, KT // 2, 2, H, 64], FP8, name="V_aug")
    nc.gpsimd.memset(V_aug[:, :, :, :, :32], 1.0)
    woT_bf = persist.tile([P, DO, D], BF16, name="woT_bf")

    def load_wT(name, dst):
        # 4 transposes into one psum tile, one rearranged bf16 copy out
        w_sb = w_nat[name]
        ps = ps_s.tile([P, 4, P], F32, tag="s")
        for o in range(DO):
            for fo in range(DO):
                transpose_f32(ps[:, 2 * o + fo, :],
                               w_sb[:, o, fo * P:(fo + 1) * P])
        nc.scalar.copy(
            dst[:].rearrange("p f (o c) -> p f o c", o=DO),
            ps.rearrange("p (o f) c -> p f o c", o=DO))
        return dst

    wkT = load_wT("Wk", ph0.tile([P, DO, D], BF16, name="WkT"))
    wqT = load_wT("Wq", ph0.tile([P, DO, D], BF16, name="WqT"))

    # transposed inputs (fp32); psum->sbuf copies split between ACT and DVE
    q_T = ph0.tile([P, DO, NQ], BF16, name="q_T")
    k_T = ph0.tile([P, DO, NK], BF16, name="k_T")

    def transpose_in(src, dst, T, t_base=0, dve_only=False):
        # per tile-pair: 4 transposes into one psum tile, one copy out
        for t0 in range(t_base, t_base + T, 2):
            ps = ps_s.tile([P, 4, P], F32, tag="s")
            for dt in range(2):
                for o in range(DO):
                    transpose_f32(ps[:, 2 * dt + o, :],
                                   src[:, t0 + dt, o * P:(o + 1) * P])
            d = dst[:, :, t0 * P:(t0 + 2) * P].rearrange(
                "p o (t c) -> p o t c", t=2)
            s = ps.rearrange("p (t o) c -> p o t c", o=DO)
            if (t0 // 2) % 2 and not dve_only:
                nc.scalar.copy(d, s)
            else:
                nc.vector.tensor_copy(d, s)

    def project(wT, src, dst, b_pm, N, bias_eng):
        for o in range(DO):
            for c0 in range(0, N, 512):
                w = min(512, N - c0)
                ps = ps_f.tile([P, 512], F32, tag="f")
                for ki in range(DO):
                    nc.tensor.matmul(
                        ps[:, :w],
                        lhsT=wT[:, ki, o * P:(o + 1) * P],
                        rhs=src[:, ki, c0:c0 + w],
                        start=(ki == 0), stop=(ki == DO - 1))
                if bias_eng == "act":
                    nc.scalar.add(dst[:, o, c0:c0 + w], ps[:, :w],
                                  b_pm[:, o:o + 1])
                else:
                    nc.vector.tensor_scalar_add(
                        dst[:, o, c0:c0 + w], ps[:, :w], b_pm[:, o:o + 1])

    def vproj(t):
        ps = ps_f.tile([P, 512], F32, tag="f")
        for ki in range(DO):
            nc.tensor.matmul(
                ps[:, :D], lhsT=k_T[:, ki, t * P:(t + 1) * P],
                rhs=wvT[:, ki, :],
                start=(ki == 0), stop=(ki == DO - 1))
        nc.vector.tensor_tensor(
            V_aug[:, t // 2, t % 2, :, 32:],
            ps[:, :D].rearrange("p (h w) -> p h w", h=H),
            B_v.rearrange("p (h w) -> p h w", h=H), OP.add)

    # k-side work interleaved per 4-tile (512-key) chunk so the first score
    # matmuls only wait for the first k DMA quarter + its proj.
    def ktrans_kproj(c, dve_only=False):
        # dve_only keeps ACT out of the K path for chunks emitted inside
        # the first attention chunk's g-loop (an ACT-side bias would sit
        # in the exp stream's critical loop)
        transpose_in(k_sb, k_T, 4, t_base=4 * c, dve_only=dve_only)
        for o in range(DO):
            ps = ps_f.tile([P, 512], F32, tag="f")
            for ki in range(DO):
                nc.tensor.matmul(
                    ps, lhsT=wkT[:, ki, o * P:(o + 1) * P],
                    rhs=k_T[:, ki, 512 * c:512 * (c + 1)],
                    start=(ki == 0), stop=(ki == DO - 1))
            if dve_only:
                nc.vector.tensor_scalar_add(
                    K_bf[:, o, 512 * c:512 * (c + 1)], ps, bk_pm[:, o:o + 1])
            else:
                nc.scalar.add(K_bf[:, o, 512 * c:512 * (c + 1)], ps,
                              bk_pm[:, o:o + 1])

    transpose_in(q_sb, q_T, QT // 2)
    project(wqT, q_T, Q_bf, bq_pm, NQ // 2, "dve")
    ktrans_kproj(0)
    # Wv lands after the first k/q pieces; only vproj (emitted inside the
    # first chunk's g-loop, after the first score group) depends on it
    wvT = load_wT("Wv", ph0.tile([P, DO, D], BF16, name="WvT"))
    B_v = ph0.tile([P, D], F32, name="B_v")
    nc.gpsimd.partition_broadcast(B_v, bv_row)
    # remaining k chunks, V projections, and the Wo transpose are emitted
    # interleaved with the first attention chunk's score groups below

    # ---- fused attention + LN/FFN/LN/store, pipelined per q-chunk ----
    # per-chunk [P, DO, SC] working tiles (double-buffered via tags)
    def u32(ap):
        return ap.bitcast(U32)

    def layer_norm_chunk(src, dst, g_pm, b_pm, sc=SC):
        """dst = LN(src) * g + b over [P, DO, SC] bf16 tiles, ACT-free.

        Stats via ones-matmul on PE ([1, DO*SC] psum row), per-token math on
        DVE [1, SC] rows, then a K=1 ones-row matmul broadcasts mu / rstd
        back to all 128 partitions through psum.
        """
        x2 = ln_p.tile([P, DO, sc], BF16, tag="x2")
        nc.vector.tensor_tensor(x2, src, src, OP.mult)
        # the DO halves accumulate inside psum; sumw=1/D makes these means
        st1 = ps_f.tile([1, 512], F32, tag="f")
        st2 = ps_f.tile([1, 512], F32, tag="f")
        for o in range(DO):
            nc.tensor.matmul(st1[:, :sc], lhsT=sumw_bf, rhs=src[:, o],
                             start=(o == 0), stop=(o == DO - 1))
            nc.tensor.matmul(st2[:, :sc], lhsT=sumw_bf, rhs=x2[:, o],
                             start=(o == 0), stop=(o == DO - 1))
        mu = ln_p.tile([1, sc], F32, tag="mu")
        ve = ln_p.tile([1, sc], F32, tag="ve")
        tmp = ln_p.tile([1, sc], F32, tag="tmp")
        nc.vector.tensor_copy(mu, st1[:, :sc])
        # mu broadcast can start as soon as mu is ready
        bc_mu = ps_f.tile([P, sc], F32, tag="f")
        nc.tensor.matmul(bc_mu, lhsT=ones_row[:], rhs=mu[:])
        nc.vector.tensor_tensor(tmp, mu, mu, OP.mult)
        nc.vector.tensor_tensor(ve, st2[:, :sc], tmp, OP.subtract)
        # y ~= rsqrt(ve): bit-trick + 1 Newton step (max rel err ~2e-3)
        y = ln_p.tile([1, sc], F32, tag="y")
        nc.vector.tensor_scalar(u32(y), u32(ve), 1, None,
                                OP.logical_shift_right)
        nc.vector.tensor_tensor(u32(y), c_magic[:, :sc], u32(y), OP.subtract)
        nc.vector.tensor_tensor(tmp, y, y, OP.mult)
        nc.vector.tensor_tensor(tmp, tmp, ve, OP.mult)
        nc.vector.tensor_scalar(tmp, tmp, -0.5, 1.5, OP.mult, OP.add)
        rstd = ln_p.tile([1, sc], F32, tag="rstd")
        nc.vector.tensor_tensor(rstd, y, tmp, OP.mult)
        bc_rs = ps_f.tile([P, sc], F32, tag="f")
        nc.tensor.matmul(bc_rs, lhsT=ones_row[:], rhs=rstd[:])
        # xm[o] = src - mu (overlaps the rsqrt chain)
        xm = ln_p.tile([P, DO, sc], BF16, tag="xm")
        for o in range(DO):
            nc.vector.tensor_tensor(xm[:, o], src[:, o], bc_mu, OP.subtract)
        # dst = ((xm * g) * rstd) + b
        for o in range(DO):
            nc.vector.scalar_tensor_tensor(
                dst[:, o], in0=xm[:, o], scalar=g_pm[:, o:o + 1], in1=bc_rs,
                op0=OP.mult, op1=OP.mult)
            nc.vector.tensor_scalar_add(dst[:, o], dst[:, o],
                                        b_pm[:, o:o + 1])

    def emit_tail(q0, sc, O_bf, last):
        # -- LN0 / FFN / LN1 / transpose+store for one chunk --
        X0 = big.tile([P, DO, sc], BF16, tag="X0")
        X1 = big.tile([P, DO, sc], BF16, tag="X1")
        X2 = big.tile([P, DO, sc], F32, tag="X2")
        out_sb = big.tile([P, sc // P, D], F32, tag="out")
        layer_norm_chunk(O_bf, X0, g0_pm, b0_pm, sc)
        for o in range(DO):
            ps = ps_f.tile([P, 512], F32, tag="f")
            for ki in range(DO):
                nc.tensor.matmul(ps[:, :sc],
                                 lhsT=woT_bf[:, ki, o * P:(o + 1) * P],
                                 rhs=X0[:, ki, :],
                                 start=(ki == 0), stop=(ki == DO - 1))
            ht = ln_p.tile([P, sc], BF16, tag="ht")
            if last:   # ACT is idle after the final exp
                nc.scalar.activation(ht, ps[:, :sc], AF.Relu,
                                     bias=bo_pm[:, o:o + 1])
            else:
                nc.vector.scalar_tensor_tensor(
                    ht, in0=ps[:, :sc], scalar=bo_pm[:, o:o + 1],
                    in1=zerot.to_broadcast([P, sc]), op0=OP.add, op1=OP.max)
            nc.vector.tensor_add(X1[:, o], X0[:, o], ht)
        layer_norm_chunk(X1, X2, g1_pm, b1_pm, sc)
        tpq = sc // P
        out_io = io["out"][:].rearrange("(t p) d -> p t d", p=P)
        t0 = q0 // P
        for tt in range(tpq):
            for o in range(DO):
                ps = ps_f.tile([P, P], F32, tag="f")
                nc.tensor.matmul(ps, lhsT=X2[:, o, tt * P:(tt + 1) * P],
                                 rhs=ident[:], is_transpose=True)
                d = out_sb[:, tt, o * P:(o + 1) * P]
                if last:
                    nc.scalar.copy(d, ps)
                else:
                    nc.vector.tensor_copy(d, ps)
            nc.sync.dma_start(out_io[:, t0 + tt:t0 + tt + 1, :],
                              out_sb[:, tt:tt + 1, :])

    pending = None
    # chunk list: three 256-query chunks, then two 128s (a short final
    # chunk shortens the post-attention dependency chain; exp-instr size
    # stays constant via KGc so ACT cost is unchanged)
    sizes = [SC] * (NQ // SC - 1) + [SC // 2, SC // 2]
    chunks = []
    cq0 = 0
    for csc in sizes:
        chunks.append((cq0, csc))
        cq0 += csc
    assert cq0 == NQ
    def scores_exp(at, h, g, KGc, sc, qsl):
        m, half = h % 4, h // 4
        ps = ps_s.tile([P, KGc, sc], F32, tag="s")
        for j in range(KGc):
            kt = g * KGc + j
            nc.tensor.matmul(
                ps[:, j, :],
                lhsT=K_bf[32 * m:32 * m + 32, half, kt * P:(kt + 1) * P],
                rhs=Q_bf[32 * m:32 * m + 32, half, qsl],
                start=True, stop=True, tile_position=(32 * m, 0))
        nc.scalar.activation(at[:, g * KGc:(g + 1) * KGc, :], ps,
                             AF.Exp, scale=1.0 / 16.0)

    for ci, (q0, sc) in enumerate(chunks):
        qsl = slice(q0, q0 + sc)
        KGc = KG * SC // sc          # keep exp instrs at KGc*sc = 1024 elems
        attn = [attn_pool.tile([P, KT, sc], FP8, tag=f"attn{h}",
                               name=f"attn{h}")
                for h in range(H)]
        # -- PV fp8 DoubleRow per 4-head wave (wave == output half).
        # Each head owns a [64, sc] slot at PE column 0; denominator on
        # partitions 0:32, numerator on 32:64. Wave A is emitted as soon as
        # heads 0-3 finish their exp (mid-chunk), which frees their attn
        # tiles for the next chunk and shortens the final-chunk chain. --
        O_bf = big.tile([P, DO, sc], BF16, tag="O")

        def pv_norm(h0, nh):
            # PV + normalize for heads h0..h0+nh (all within one output
            # half). A smaller final wave shrinks the PE work a chunk
            # boundary has to carry before the next chunk's scores.
            half, m0 = h0 // 4, h0 % 4
            pot = ps_o.tile([P, 4, sc], F32, tag="po")
            for j in range(nh):
                h = h0 + j
                for tp in range(KT // 2):
                    nc.tensor.matmul(
                        pot[0:64, j, :],
                        lhsT=V_aug[:, tp, :, h, :],
                        rhs=attn[h][:, 2 * tp:2 * tp + 2, :],
                        start=(tp == 0), stop=(tp == KT // 2 - 1),
                        perf_mode=DR, skip_group_check=True)
            rec = rec_p.tile([P, 4, sc], F32, tag="rec")
            nc.vector.reciprocal(rec[32:64, :nh], pot[0:32, :nh])
            S = rec_p.tile([P, sc], BF16, tag="S")
            for j in range(nh):
                m = m0 + j
                nc.vector.tensor_tensor(
                    S[32 * m:32 * m + 32, :], pot[32:64, j, :],
                    rec[32:64, j, :], OP.mult)
            nc.vector.tensor_add(
                O_bf[32 * m0:32 * (m0 + nh), half, :],
                S[32 * m0:32 * (m0 + nh), :],
                Q_bf[32 * m0:32 * (m0 + nh), half, qsl])

        if ci == 0:
            # g-major, interleaved with the remaining k-side pipeline: the
            # exp stream starts right after the first k quarter's projection.
            # Each score group is followed only by the NEXT k chunk's
            # transpose+projection; V/Wo/q-half-2 slot in per group.
            G = KT // KGc
            for g in range(G):
                for h in range(H):
                    scores_exp(attn[h], h, g, KGc, sc, qsl)
                if g + 1 < G:
                    ktrans_kproj(g + 1, dve_only=True)
                for t in range(KGc * g, KGc * g + KGc):
                    vproj(t)
                if g == 0:   # second q half lands after k quarter 2
                    transpose_in(q_sb, q_T, QT // 2, t_base=QT // 2)
                    project(wqT, q_T[:, :, NQ // 2:],
                            Q_bf[:, :, NQ // 2:], bq_pm, NQ // 2, "dve")
            load_wT("Wo", woT_bf)
            ph0_cm.__exit__(None, None, None)
            pv_norm(0, 4)
            pv_norm(4, 2)
            pv_norm(6, 1)
            pv_norm(7, 1)
        else:
            mid_pv = ci == len(chunks) - 1   # no exp left to starve
            for h in range(H):
                if h == 4 and mid_pv:
                    pv_norm(0, 4)
                for g in range(KT // KGc):
                    scores_exp(attn[h], h, g, KGc, sc, qsl)
            if mid_pv:
                # tapered waves: all but the last become executable before
                # the final exp, so the terminal chain carries 1 head only
                pv_norm(4, 2)
                pv_norm(6, 1)
                pv_norm(7, 1)
            else:
                pv_norm(0, 4)
                pv_norm(4, 2)
                pv_norm(6, 1)
                pv_norm(7, 1)
        # tail (LN0/FFN/LN1/store) is deferred until after the NEXT
        # chunk's scores are emitted, so the chunk boundary on PE carries
        # only PV wave B and the exp stream barely pauses
        if pending is not None:
            emit_tail(*pending)
        pending = (q0, sc, O_bf, ci == len(chunks) - 1)
    emit_tail(*pending)


def build(cfg: Cfg) -> bass.Bass:
    nc = bacc.Bacc("TRN2")
    io = {}
    for name, shape in (
        ("q", [cfg.NQ, cfg.D]), ("k", [cfg.NK, cfg.D]),
        ("Wq", [cfg.D, cfg.D]), ("Wk", [cfg.D, cfg.D]),
        ("Wv", [cfg.D, cfg.D]), ("Wo", [cfg.D, cfg.D]),
        ("bq", [cfg.D]), ("bk", [cfg.D]), ("bv", [cfg.D]), ("bo", [cfg.D]),
        ("g0", [cfg.D]), ("b0", [cfg.D]), ("g1", [cfg.D]), ("b1", [cfg.D]),
    ):
        io[name] = nc.dram_tensor(name, shape, F32, kind="ExternalInput")
    io["out"] = nc.dram_tensor("out", [cfg.NQ, cfg.D], F32, kind="ExternalOutput")

    with tile.TileContext(nc) as tc:
        with ExitStack() as ctx:
            _emit(nc, tc, ctx, io, cfg)
    nc.compile()
    return nc


_CACHE = {}


def _get_nc(key, cfg):
    if key not in _CACHE:
        _CACHE[key] = build(cfg)
    return _CACHE[key]


def kernel(q, k, Wq, bq, Wk, bk, Wv, bv, Wo, bo, g0, b0, g1, b1, _trace=False):
    from concourse.bass_utils import run_bass_kernel_spmd

    B, Nq, D = q.shape
    Nk = k.shape[1]
    n_cores = 8
    halves = n_cores // B
    nq_c = Nq // halves
    cfg = Cfg(NQ=nq_c, NK=Nk, D=D)
    nc = _get_nc((nq_c, Nk, D), cfg)

    shared = dict(Wq=Wq, bq=bq, Wk=Wk, bk=bk, Wv=Wv, bv=bv, Wo=Wo, bo=bo,
                  g0=g0, b0=b0, g1=g1, b1=b1)
    shared = {n: np.ascontiguousarray(v, dtype=np.float32)
              for n, v in shared.items()}
    in_maps = []
    for c in range(n_cores):
        b, hf = c // halves, c % halves
        m = dict(shared)
        m["q"] = np.ascontiguousarray(q[b, hf * nq_c:(hf + 1) * nq_c], np.float32)
        m["k"] = np.ascontiguousarray(k[b], np.float32)
        in_maps.append(m)

    res = run_bass_kernel_spmd(nc, in_maps, core_ids=list(range(n_cores)),
                               trace=_trace)
    out = np.empty((B, Nq, D), np.float32)
    for c in range(n_cores):
        b, hf = c // halves, c % halves
        out[b, hf * nq_c:(hf + 1) * nq_c] = res.results[c]["out"]
    if _trace:
        return out, res
    return out


# revision 72
# speedup vs baseline: 1.0678x; 1.0678x over previous
"""MAB (Set-Transformer multihead attention block) Trainium2 Bass kernel, v4.

Reference math (fp32):
  Q = q @ Wq.T + bq ; K = k @ Wk.T + bk ; V = k @ Wv.T + bv    [B,N,256]
  per head h (8 heads x 32): s = Qh @ Kh.T / 16 ; a = softmax(s)
  Oh = Qh + a @ Vh ; o = concat(Oh) ; o = LN0(o) ; o = o + relu(o @ Wo.T + bo)
  out = LN1(o)

Sharding: 8 cores = (batch b in 0..3, query-half in 0..1). Each core handles
1024 queries x 2048 keys of one batch; rows are fully independent through
the whole block (LN/FFN are per-row), so there are no collectives.

Design notes (driven by the TimelineSim cost model; ACT-exp is the
bottleneck engine at ~133us/core and everything else hides under it —
total ~181us/core vs a 457us all-fp32 baseline):
  - score matmuls: direct K=32 row-tiled bf16 contraction (Matmult engine
    time depends only on out-free-rows, not K/M — no head replication).
  - PV runs fp8e4 DoubleRow (0.5 cyc/row, 2 k-tiles contracted per
    instruction; must sit at PE column 0 with M<=64): V_aug carries
    per-head [ones(32) | V(32)] so one accumulation yields the softmax
    denominator (replicated over psum partitions 0:32) AND the numerator
    (32:64). PV goes in two 4-head waves; wave == transposed output half.
  - exp on ACT reads score-PSUM [128, 1024] and writes the fp8 attn tile
    directly; the table is preloaded via a dummy activation; all Q/K/V/W
    matmul inputs are bf16 (fp32r would require fp32r-rounded producers,
    which DMA-fed tensors cannot provide).
  - LN is ACT-free so it never thrashes the exp activation table: stats
    via 1/D-ones matmuls on PE (halves accumulated in psum), per-token
    row math + fast-inverse-sqrt (bit trick + 1 Newton, 0.2% max err) on
    DVE, then K=1 ones-row matmuls broadcast mu/rstd through psum.
    The LN0+FFN+LN1+transpose-out tail of each chunk is emitted after
    the NEXT chunk's scores so chunk boundaries only carry PV wave B.
  - query chunks 3x256 + 2x128 (the short last chunk shrinks the final
    dependency chain); exp instruction size is held at 1024 elements.
  - ALL psum comes from three always-open pools (8 banks total) with
    shared tags, so there is no pool-scope barrier between the preamble
    and the attention loop.
  - ALL input DMAs ride one queue, ordered Wk, Wq, q-half-1, bq/bk,
    k-quarter-1, ... so the first score matmul fires ~10us in; the first
    chunk runs its score groups g-major, interleaved with the remaining
    k-quarter transposes/projections as they land.
"""

import os
import sys
from contextlib import ExitStack

import numpy as np

for _p in ("/opt/trn_rl_repo", "/root/.axon_site/_ro/trn_rl_repo"):
    if os.path.isdir(_p) and _p not in sys.path:
        sys.path.insert(0, _p)

import concourse.bass as bass  # noqa: E402
import concourse.tile as tile  # noqa: E402
from concourse import bacc, bass_isa, mybir  # noqa: E402
from concourse.masks import make_identity  # noqa: E402

F32 = mybir.dt.float32
F32R = mybir.dt.float32r
BF16 = mybir.dt.bfloat16
FP8 = mybir.dt.float8e4
U32 = mybir.dt.uint32
P = 128
EPS = 1e-5

AF = mybir.ActivationFunctionType
OP = mybir.AluOpType
DR = mybir.MatmulPerfMode.DoubleRow
RADD = bass_isa.ReduceOp.add


class Cfg:
    def __init__(self, NQ=1024, NK=2048, D=256, H=8, SC=256):
        self.NQ, self.NK, self.D, self.H = NQ, NK, D, H
        self.HD = D // H            # 32
        self.DO = D // P            # 2
        self.QT = NQ // P
        self.KT = NK // P
        self.SC = min(SC, NQ)       # attention + ln/ffn q-chunk
        self.QCN = NQ // self.SC
        assert self.HD == 32 and self.DO == 2
        assert self.KT % 2 == 0 and self.SC % P == 0


def _emit(nc: bass.Bass, tc: tile.TileContext, ctx: ExitStack, io: dict, cfg: Cfg):
    NQ, NK, D, H = cfg.NQ, cfg.NK, cfg.D, cfg.H
    DO, QT, KT, SC = cfg.DO, cfg.QT, cfg.KT, cfg.SC
    KG = 4                       # score k-tiles per psum tile / exp call

    const = ctx.enter_context(tc.tile_pool(name="const", bufs=1))
    persist = ctx.enter_context(tc.tile_pool(name="persist", bufs=1))
    big = ctx.enter_context(tc.tile_pool(name="big", bufs=2))
    # single set of psum pools for the WHOLE kernel (8 banks):
    #   ps_s : tag "s"  2x[P,KG,SC]f32  (4 banks) scores / transposes
    #   ps_o : tags po0/po1 1x[P,2,SC]f32 each (2 banks) PV accum
    #   ps_f : tag "f"  2x[P,512]f32    (2 banks) projections / FFN / out-T
    ps_s = ctx.enter_context(tc.tile_pool(name="ps_s", bufs=2, space="PSUM"))
    ps_o = ctx.enter_context(tc.tile_pool(name="ps_o", bufs=1, space="PSUM"))
    ps_f = ctx.enter_context(tc.tile_pool(name="ps_f", bufs=2, space="PSUM"))
    attn_pool = ctx.enter_context(tc.tile_pool(name="attn", bufs=1))
    rec_p = ctx.enter_context(tc.tile_pool(name="rec_p", bufs=2))
    ln_p = ctx.enter_context(tc.tile_pool(name="ln_p", bufs=2))

    # ---- input DMAs first. The DMA engines are a single shared resource
    # (~330 GB/s effective), so ORDER is what matters: small weights and q
    # first (unblocks the Q-side pipeline), then k in halves (k_T transposes
    # of the first half overlap the second half's transfer), then Wv/Wo.
    # ALL input DMAs ride one queue (SP) so the shared DMA engine serves
    # them in exactly this order: Wk, Wq, first k quarter, first q half,
    # biases, Wv, k2, q2, k3, k4, Wo. The first score matmul needs only
    # {Wk, k quarter 1, Wq, q half 1}.
    ph0_cm = tc.tile_pool(name="ph0", bufs=1)
    ph0 = ph0_cm.__enter__()
    w_nat = {}

    def load_w_nat(name):
        w_nat[name] = ph0.tile([P, DO, D], F32, name=f"{name}_nat")
        nc.sync.dma_start(w_nat[name],
                          io[name][:].rearrange("(o p) f -> p o f", p=P))

    def vec_pm(name):
        t = const.tile([P, DO], F32, name=f"{name}_pm")
        nc.sync.dma_start(t, io[name][:].rearrange("(o p) -> p o", p=P))
        return t

    load_w_nat("Wk")
    load_w_nat("Wq")
    q_sb = ph0.tile([P, QT, D], F32, name="q_nat")
    qio = io["q"][:].rearrange("(t p) d -> p t d", p=P)
    nc.sync.dma_start(q_sb[:, :QT // 2, :], qio[:, :QT // 2, :])
    bq_pm, bk_pm = vec_pm("bq"), vec_pm("bk")
    k_sb = ph0.tile([P, KT, D], F32, name="k_nat")
    kio = io["k"][:].rearrange("(t p) d -> p t d", p=P)
    nc.sync.dma_start(k_sb[:, 0:4, :], kio[:, 0:4, :])

    bv_row = const.tile([1, D], F32, name="bv_row")
    nc.sync.dma_start(bv_row, io["bv"][:].rearrange("(o d) -> o d", o=1))
    bo_pm = vec_pm("bo")
    g0_pm, b0_pm = vec_pm("g0"), vec_pm("b0")
    g1_pm, b1_pm = vec_pm("g1"), vec_pm("b1")
    load_w_nat("Wv")
    nc.sync.dma_start(k_sb[:, 4:8, :], kio[:, 4:8, :])
    nc.sync.dma_start(q_sb[:, QT // 2:, :], qio[:, QT // 2:, :])
    for c in range(2, KT // 4):
        nc.sync.dma_start(k_sb[:, 4 * c:4 * (c + 1), :],
                          kio[:, 4 * c:4 * (c + 1), :])
    load_w_nat("Wo")

    # ---- constants ----
    ident = const.tile([P, P], F32)
    make_identity(nc, ident)
    ident_bf = const.tile([P, P], BF16)
    nc.vector.tensor_copy(ident_bf, ident)
    zerot = const.tile([P, 1], F32)
    nc.vector.memset(zerot, 0.0)
    sumw_bf = const.tile([P, 1], BF16)   # 1/D: stats matmuls yield means
    nc.vector.memset(sumw_bf, 1.0 / D)
    ones_row = const.tile([1, P], BF16)
    nc.vector.memset(ones_row, 1.0)
    c_magic = const.tile([1, SC], U32)
    nc.vector.memset(c_magic, 0x5F3759DF)
    # preload the exp activation table while ACT is otherwise idle
    scratch1 = const.tile([P, 1], F32)
    nc.scalar.activation(scratch1, zerot, AF.Exp)

    def transpose_f32(ps, src):
        nc.tensor.matmul(ps, lhsT=src, rhs=ident[:], is_transpose=True)

    # ---- phase 0: weight/input transposes + projections ----
    # V_aug per head h: 64 fp8 columns = [ones(32) | V(32)]. The DoubleRow
    # PV (which must sit at PE column 0, M<=64) then yields the softmax
    # denominator on psum partitions 0:32 and the numerator on 32:64.
    Q_bf = persist.tile([P, DO, NQ], BF16, name="Q_bf")
    K_bf = persist.tile([P, DO, NK], BF16, name="K_bf")
    V_aug = persist.tile([P, KT // 2, 2, H, 64], FP8, name="V_aug")
    nc.gpsimd.memset(V_aug[:, :, :, :, :32], 1.0)
    woT_bf = persist.tile([P, DO, D], BF16, name="woT_bf")

    def load_wT(name, dst):
        # 4 transposes into one psum tile, one rearranged bf16 copy out
        w_sb = w_nat[name]
        ps = ps_f.tile([P, 4, P], F32, tag="f")
        for o in range(DO):
            for fo in range(DO):
                transpose_f32(ps[:, 2 * o + fo, :],
                               w_sb[:, o, fo * P:(fo + 1) * P])
        nc.scalar.copy(
            dst[:].rearrange("p f (o c) -> p f o c", o=DO),
            ps.rearrange("p (o f) c -> p f o c", o=DO))
        return dst

    wkT = load_wT("Wk", ph0.tile([P, DO, D], BF16, name="WkT"))
    wqT = load_wT("Wq", ph0.tile([P, DO, D], BF16, name="WqT"))

    # transposed inputs (fp32); psum->sbuf copies split between ACT and DVE
    q_T = ph0.tile([P, DO, NQ], BF16, name="q_T")
    k_T = ph0.tile([P, DO, NK], BF16, name="k_T")

    def transpose_in(src, dst, T, t_base=0, dve_only=False):
        # per tile-pair: 4 transposes into one psum tile, one copy out
        for t0 in range(t_base, t_base + T, 2):
            ps = ps_f.tile([P, 4, P], F32, tag="f")
            for dt in range(2):
                for o in range(DO):
                    transpose_f32(ps[:, 2 * dt + o, :],
                                   src[:, t0 + dt, o * P:(o + 1) * P])
            d = dst[:, :, t0 * P:(t0 + 2) * P].rearrange(
                "p o (t c) -> p o t c", t=2)
            s = ps.rearrange("p (t o) c -> p o t c", o=DO)
            if (t0 // 2) % 2 and not dve_only:
                nc.scalar.copy(d, s)
            else:
                nc.vector.tensor_copy(d, s)

    def project(wT, src, dst, b_pm, N, bias_eng):
        for o in range(DO):
            for c0 in range(0, N, 512):
                w = min(512, N - c0)
                ps = ps_f.tile([P, 512], F32, tag="f")
                for ki in range(DO):
                    nc.tensor.matmul(
                        ps[:, :w],
                        lhsT=wT[:, ki, o * P:(o + 1) * P],
                        rhs=src[:, ki, c0:c0 + w],
                        start=(ki == 0), stop=(ki == DO - 1))
                if bias_eng == "act":
                    nc.scalar.add(dst[:, o, c0:c0 + w], ps[:, :w],
                                  b_pm[:, o:o + 1])
                else:
                    nc.vector.tensor_scalar_add(
                        dst[:, o, c0:c0 + w], ps[:, :w], b_pm[:, o:o + 1])

    def vproj(t):
        ps = ps_f.tile([P, 512], F32, tag="f")
        for ki in range(DO):
            nc.tensor.matmul(
                ps[:, :D], lhsT=k_T[:, ki, t * P:(t + 1) * P],
                rhs=wvT[:, ki, :],
                start=(ki == 0), stop=(ki == DO - 1))
        nc.vector.tensor_tensor(
            V_aug[:, t // 2, t % 2, :, 32:],
            ps[:, :D].rearrange("p (h w) -> p h w", h=H),
            B_v.rearrange("p (h w) -> p h w", h=H), OP.add)

    # k-side work interleaved per 4-tile (512-key) chunk so the first score
    # matmuls only wait for the first k DMA quarter + its proj.
    def ktrans_kproj(c, dve_only=False):
        # dve_only keeps ACT out of the K path for chunks emitted inside
        # the first attention chunk's g-loop (an ACT-side bias would sit
        # in the exp stream's critical loop)
        transpose_in(k_sb, k_T, 4, t_base=4 * c, dve_only=dve_only)
        for o in range(DO):
            ps = ps_f.tile([P, 512], F32, tag="f")
            for ki in range(DO):
                nc.tensor.matmul(
                    ps, lhsT=wkT[:, ki, o * P:(o + 1) * P],
                    rhs=k_T[:, ki, 512 * c:512 * (c + 1)],
                    start=(ki == 0), stop=(ki == DO - 1))
            if dve_only:
                nc.vector.tensor_scalar_add(
                    K_bf[:, o, 512 * c:512 * (c + 1)], ps, bk_pm[:, o:o + 1])
            else:
                nc.scalar.add(K_bf[:, o, 512 * c:512 * (c + 1)], ps,
                              bk_pm[:, o:o + 1])

    transpose_in(q_sb, q_T, QT // 2)
    project(wqT, q_T, Q_bf, bq_pm, NQ // 2, "dve")
    ktrans_kproj(0)
    # Wv lands after the first k/q pieces; only vproj (emitted inside the
    # first chunk's g-loop, after the first score group) depends on it
    wvT = load_wT("Wv", ph0.tile([P, DO, D], BF16, name="WvT"))
    B_v = ph0.tile([P, D], F32, name="B_v")
    nc.gpsimd.partition_broadcast(B_v, bv_row)
    # remaining k chunks, V projections, and the Wo transpose are emitted
    # interleaved with the first attention chunk's score groups below

    # ---- fused attention + LN/FFN/LN/store, pipelined per q-chunk ----
    # per-chunk [P, DO, SC] working tiles (double-buffered via tags)
    def u32(ap):
        return ap.bitcast(U32)

    def layer_norm_chunk(src, dst, g_pm, b_pm, sc=SC):
        """dst = LN(src) * g + b over [P, DO, SC] bf16 tiles, ACT-free.

        Stats via ones-matmul on PE ([1, DO*SC] psum row), per-token math on
        DVE [1, SC] rows, then a K=1 ones-row matmul broadcasts mu / rstd
        back to all 128 partitions through psum.
        """
        x2 = ln_p.tile([P, DO, sc], BF16, tag="x2")
        nc.vector.tensor_tensor(x2, src, src, OP.mult)
        # the DO halves accumulate inside psum; sumw=1/D makes these means
        st1 = ps_f.tile([1, 512], F32, tag="f")
        st2 = ps_f.tile([1, 512], F32, tag="f")
        for o in range(DO):
            nc.tensor.matmul(st1[:, :sc], lhsT=sumw_bf, rhs=src[:, o],
                             start=(o == 0), stop=(o == DO - 1))
            nc.tensor.matmul(st2[:, :sc], lhsT=sumw_bf, rhs=x2[:, o],
                             start=(o == 0), stop=(o == DO - 1))
        mu = ln_p.tile([1, sc], BF16, tag="mu")
        ve = ln_p.tile([1, sc], F32, tag="ve")
        tmp = ln_p.tile([1, sc], F32, tag="tmp")
        nc.vector.tensor_copy(mu, st1[:, :sc])
        # mu broadcast can start as soon as mu is ready
        bc_mu = ps_f.tile([P, sc], F32, tag="f")
        nc.tensor.matmul(bc_mu, lhsT=ones_row[:], rhs=mu[:])
        nc.vector.tensor_tensor(tmp, mu, mu, OP.mult)
        nc.vector.tensor_tensor(ve, st2[:, :sc], tmp, OP.subtract)
        # y ~= rsqrt(ve): bit-trick + 1 Newton step (max rel err ~2e-3)
        y = ln_p.tile([1, sc], F32, tag="y")
        nc.vector.tensor_scalar(u32(y), u32(ve), 1, None,
                                OP.logical_shift_right)
        nc.vector.tensor_tensor(u32(y), c_magic[:, :sc], u32(y), OP.subtract)
        nc.vector.tensor_tensor(tmp, y, y, OP.mult)
        nc.vector.tensor_tensor(tmp, tmp, ve, OP.mult)
        nc.vector.tensor_scalar(tmp, tmp, -0.5, 1.5, OP.mult, OP.add)
        rstd = ln_p.tile([1, sc], BF16, tag="rstd")
        nc.vector.tensor_tensor(rstd, y, tmp, OP.mult)
        bc_rs = ps_f.tile([P, sc], F32, tag="f")
        nc.tensor.matmul(bc_rs, lhsT=ones_row[:], rhs=rstd[:])
        # xm[o] = src - mu (overlaps the rsqrt chain)
        xm = ln_p.tile([P, DO, sc], BF16, tag="xm")
        for o in range(DO):
            nc.vector.tensor_tensor(xm[:, o], src[:, o], bc_mu, OP.subtract)
        # dst = ((xm * g) * rstd) + b
        for o in range(DO):
            nc.vector.scalar_tensor_tensor(
                dst[:, o], in0=xm[:, o], scalar=g_pm[:, o:o + 1], in1=bc_rs,
                op0=OP.mult, op1=OP.mult)
            nc.vector.tensor_scalar_add(dst[:, o], dst[:, o],
                                        b_pm[:, o:o + 1])

    def emit_tail(q0, sc, O_bf, last):
        # -- LN0 / FFN / LN1 / transpose+store for one chunk --
        X0 = big.tile([P, DO, sc], BF16, tag="X0")
        X1 = big.tile([P, DO, sc], BF16, tag="X1")
        X2 = big.tile([P, DO, sc], F32, tag="X2")
        out_sb = big.tile([P, sc // P, D], F32, tag="out")
        layer_norm_chunk(O_bf, X0, g0_pm, b0_pm, sc)
        for o in range(DO):
            ps = ps_f.tile([P, 512], F32, tag="f")
            for ki in range(DO):
                nc.tensor.matmul(ps[:, :sc],
                                 lhsT=woT_bf[:, ki, o * P:(o + 1) * P],
                                 rhs=X0[:, ki, :],
                                 start=(ki == 0), stop=(ki == DO - 1))
            ht = ln_p.tile([P, sc], BF16, tag="ht")
            if last:   # ACT is idle after the final exp
                nc.scalar.activation(ht, ps[:, :sc], AF.Relu,
                                     bias=bo_pm[:, o:o + 1])
            else:
                nc.vector.scalar_tensor_tensor(
                    ht, in0=ps[:, :sc], scalar=bo_pm[:, o:o + 1],
                    in1=zerot.to_broadcast([P, sc]), op0=OP.add, op1=OP.max)
            nc.vector.tensor_add(X1[:, o], X0[:, o], ht)
        layer_norm_chunk(X1, X2, g1_pm, b1_pm, sc)
        tpq = sc // P
        out_io = io["out"][:].rearrange("(t p) d -> p t d", p=P)
        t0 = q0 // P
        for tt in range(tpq):
            for o in range(DO):
                ps = ps_f.tile([P, P], F32, tag="f")
                nc.tensor.matmul(ps, lhsT=X2[:, o, tt * P:(tt + 1) * P],
                                 rhs=ident[:], is_transpose=True)
                d = out_sb[:, tt, o * P:(o + 1) * P]
                if last:
                    nc.scalar.copy(d, ps)
                else:
                    nc.vector.tensor_copy(d, ps)
            nc.sync.dma_start(out_io[:, t0 + tt:t0 + tt + 1, :],
                              out_sb[:, tt:tt + 1, :])

    pending = None
    # chunk list: three 256-query chunks, then two 128s (a short final
    # chunk shortens the post-attention dependency chain; exp-instr size
    # stays constant via KGc so ACT cost is unchanged)
    sizes = [SC] * (NQ // SC - 1) + [SC // 2, SC // 2]
    chunks = []
    cq0 = 0
    for csc in sizes:
        chunks.append((cq0, csc))
        cq0 += csc
    assert cq0 == NQ
    def scores_exp(at, h, g, KGc, sc, qsl):
        m, half = h % 4, h // 4
        ps = ps_s.tile([P, KGc, sc], F32, tag="s")
        for j in range(KGc):
            kt = g * KGc + j
            nc.tensor.matmul(
                ps[:, j, :],
                lhsT=K_bf[32 * m:32 * m + 32, half, kt * P:(kt + 1) * P],
                rhs=Q_bf[32 * m:32 * m + 32, half, qsl],
                start=True, stop=True, tile_position=(32 * m, 0))
        nc.scalar.activation(at[:, g * KGc:(g + 1) * KGc, :], ps,
                             AF.Exp, scale=1.0 / 16.0)

    for ci, (q0, sc) in enumerate(chunks):
        qsl = slice(q0, q0 + sc)
        KGc = KG * SC // sc          # keep exp instrs at KGc*sc = 1024 elems
        attn = [attn_pool.tile([P, KT, sc], FP8, tag=f"attn{h}",
                               name=f"attn{h}")
                for h in range(H)]
        # -- PV fp8 DoubleRow per 4-head wave (wave == output half).
        # Each head owns a [64, sc] slot at PE column 0; denominator on
        # partitions 0:32, numerator on 32:64. Wave A is emitted as soon as
        # heads 0-3 finish their exp (mid-chunk), which frees their attn
        # tiles for the next chunk and shortens the final-chunk chain. --
        O_bf = big.tile([P, DO, sc], BF16, tag="O")

        def pv_norm(h0, nh):
            # PV + normalize for heads h0..h0+nh (all within one output
            # half). A smaller final wave shrinks the PE work a chunk
            # boundary has to carry before the next chunk's scores.
            half, m0 = h0 // 4, h0 % 4
            pot = ps_o.tile([P, 4, sc], F32, tag="po")
            for j in range(nh):
                h = h0 + j
                for tp in range(KT // 2):
                    nc.tensor.matmul(
                        pot[0:64, j, :],
                        lhsT=V_aug[:, tp, :, h, :],
                        rhs=attn[h][:, 2 * tp:2 * tp + 2, :],
                        start=(tp == 0), stop=(tp == KT // 2 - 1),
                        perf_mode=DR, skip_group_check=True)
            rec = rec_p.tile([P, 4, sc], F32, tag="rec")
            nc.vector.reciprocal(rec[32:64, :nh], pot[0:32, :nh])
            S = rec_p.tile([P, sc], BF16, tag="S")
            for j in range(nh):
                m = m0 + j
                nc.vector.tensor_tensor(
                    S[32 * m:32 * m + 32, :], pot[32:64, j, :],
                    rec[32:64, j, :], OP.mult)
            nc.vector.tensor_add(
                O_bf[32 * m0:32 * (m0 + nh), half, :],
                S[32 * m0:32 * (m0 + nh), :],
                Q_bf[32 * m0:32 * (m0 + nh), half, qsl])

        if ci == 0:
            # g-major, interleaved with the remaining k-side pipeline: the
            # exp stream starts right after the first k quarter's projection.
            # Each score group is followed only by the NEXT k chunk's
            # transpose+projection; V/Wo/q-half-2 slot in per group.
            G = KT // KGc
            for g in range(G):
                for h in range(H):
                    scores_exp(attn[h], h, g, KGc, sc, qsl)
                if g + 1 < G:
                    ktrans_kproj(g + 1, dve_only=True)
                for t in range(KGc * g, KGc * g + KGc):
                    vproj(t)
                if g == 0:   # second q half lands after k quarter 2
                    transpose_in(q_sb, q_T, QT // 2, t_base=QT // 2)
                    project(wqT, q_T[:, :, NQ // 2:],
                            Q_bf[:, :, NQ // 2:], bq_pm, NQ // 2, "dve")
            load_wT("Wo", woT_bf)
            ph0_cm.__exit__(None, None, None)
            pv_norm(0, 4)
            pv_norm(4, 2)
            pv_norm(6, 1)
            pv_norm(7, 1)
        else:
            mid_pv = ci == len(chunks) - 1   # no exp left to starve
            for h in range(H):
                if h == 4 and mid_pv:
                    pv_norm(0, 4)
                for g in range(KT // KGc):
                    scores_exp(attn[h], h, g, KGc, sc, qsl)
            if mid_pv:
                # tapered waves: all but the last become executable before
                # the final exp, so the terminal chain carries 1 head only
                pv_norm(4, 2)
                pv_norm(6, 1)
                pv_norm(7, 1)
            else:
                pv_norm(0, 4)
                pv_norm(4, 2)
                pv_norm(6, 1)
                pv_norm(7, 1)
        # tail (LN0/FFN/LN1/store) is deferred until after the NEXT
        # chunk's scores are emitted, so the chunk boundary on PE carries
        # only PV wave B and the exp stream barely pauses
        if pending is not None:
            emit_tail(*pending)
        pending = (q0, sc, O_bf, ci == len(chunks) - 1)
    emit_tail(*pending)


def build(cfg: Cfg) -> bass.Bass:
    nc = bacc.Bacc("TRN2")
    io = {}
    for name, shape in (
        ("q", [cfg.NQ, cfg.D]), ("k", [cfg.NK, cfg.D]),
        ("Wq", [cfg.D, cfg.D]), ("Wk", [cfg.D, cfg.D]),
        ("Wv", [cfg.D, cfg.D]), ("Wo", [cfg.D, cfg.D]),
        ("bq", [cfg.D]), ("bk", [cfg.D]), ("bv", [cfg.D]), ("bo", [cfg.D]),
        ("g0", [cfg.D]), ("b0", [cfg.D]), ("g1", [cfg.D]), ("b1", [cfg.D]),
    ):
        io[name] = nc.dram_tensor(name, shape, F32, kind="ExternalInput")
    io["out"] = nc.dram_tensor("out", [cfg.NQ, cfg.D], F32, kind="ExternalOutput")

    with tile.TileContext(nc) as tc:
        with ExitStack() as ctx:
            _emit(nc, tc, ctx, io, cfg)
    nc.compile()
    return nc


_CACHE = {}


def _get_nc(key, cfg):
    if key not in _CACHE:
        _CACHE[key] = build(cfg)
    return _CACHE[key]


def kernel(q, k, Wq, bq, Wk, bk, Wv, bv, Wo, bo, g0, b0, g1, b1, _trace=False):
    from concourse.bass_utils import run_bass_kernel_spmd

    B, Nq, D = q.shape
    Nk = k.shape[1]
    n_cores = 8
    halves = n_cores // B
    nq_c = Nq // halves
    cfg = Cfg(NQ=nq_c, NK=Nk, D=D)
    nc = _get_nc((nq_c, Nk, D), cfg)

    shared = dict(Wq=Wq, bq=bq, Wk=Wk, bk=bk, Wv=Wv, bv=bv, Wo=Wo, bo=bo,
                  g0=g0, b0=b0, g1=g1, b1=b1)
    shared = {n: np.ascontiguousarray(v, dtype=np.float32)
              for n, v in shared.items()}
    in_maps = []
    for c in range(n_cores):
        b, hf = c // halves, c % halves
        m = dict(shared)
        m["q"] = np.ascontiguousarray(q[b, hf * nq_c:(hf + 1) * nq_c], np.float32)
        m["k"] = np.ascontiguousarray(k[b], np.float32)
        in_maps.append(m)

    res = run_bass_kernel_spmd(nc, in_maps, core_ids=list(range(n_cores)),
                               trace=_trace)
    out = np.empty((B, Nq, D), np.float32)
    for c in range(n_cores):
        b, hf = c // halves, c % halves
        out[b, hf * nq_c:(hf + 1) * nq_c] = res.results[c]["out"]
    if _trace:
        return out, res
    return out
